# revision 25
# baseline (speedup 1.0000x reference)
"""Trainium2 Bass kernel for CustomRBF forward:

    out[i] = w * exp(-gamma * ||X[i] - centroid||^2) + b

Hybrid dual-path design (per core, data-parallel over 8 cores):
  - DMA X in natural layout [128 samples (partitions), 128 feats (free)],
    16 sample-tiles (1 MB) per dma_start.
  - Half-groups of 8 tiles alternate between two compute paths so TensorE
    and VectorE each carry ~half the per-sample reduction load in parallel:
    * PE path: TensorE transpose each tile to PSUM [feat, sample]; ScalarE
      fused subtract+square (activation Square, bias=-c, per-partition =
      per-feature); TensorE fp32r matmul (squared tile stationary, [1,0]
      moving) reduces over partitions -> 2 PSUM columns per tile ([sum, 0])
      in a [128, 512] accumulator.
    * DVE path (natural layout): VectorE tensor_sub against a replicated
      centroid row, ScalarE Square, VectorE segmented tensor_reduce over
      the feature axis -> [128, 8] columns in an SBUF [128, 256] accumulator.
  - Finalize per accumulator: ScalarE Exp (scale=-gamma), VectorE
    tensor_scalar (*w + b), TensorE transpose-back in 128-col chunks,
    VectorE PSUM->SBUF copy, then one output DMA per run of consecutive
    tiles (each path owns interleaved 8-tile blocks).

Sharding: cores 0-6 take contiguous 125056-sample slices; core 7 takes the
last 125056 samples (overlapping core 6 by 448 samples so every core gets
exactly 977 full 128-sample tiles). The overlap is recomputed identically
and overwritten at gather time.

`repeats` re-emits the whole pipeline R times in one NEFF (same data, same
output) — used only for differential wall-clock timing of the steady state.
"""

import sys

sys.path.insert(0, "/opt/trn_rl_repo")

import numpy as np

D = 128          # feature dim
P = 128          # SBUF partitions
GAMMA = 1.0 / D
N_CORES = 8
TILES = 977      # 128-sample tiles per core
SHARD = TILES * P           # 125056
N_TOTAL = 1000000
GROUP = 16       # tiles per DMA
HGROUP = 8       # tiles per half-group
PO_TILES = 256   # tiles per accumulator

_NC_CACHE = {}


def _build(tiles=TILES, po_tiles=PO_TILES, repeats=1, group=GROUP,
           xin_bufs=4, y_bufs=3, tr_bufs=2, stage="full", pe_num=1, den=2):
    from contextlib import ExitStack

    import concourse.tile as tile
    from concourse import bacc, mybir

    f32 = mybir.dt.float32
    f32r = mybir.dt.float32r
    Act = mybir.ActivationFunctionType
    Alu = mybir.AluOpType

    n = tiles * P
    nc = bacc.Bacc("TRN2", target_bir_lowering=False, debug=False,
                   num_devices=N_CORES)
    xh = nc.declare_dram_parameter("x", [n, D], f32, isOutput=False)
    negch = nc.declare_dram_parameter("negc", [P, 1], f32, isOutput=False)
    identh = nc.declare_dram_parameter("ident", [P, D], f32, isOutput=False)
    onesh = nc.declare_dram_parameter("ones", [P, 2], f32, isOutput=False)
    creph = nc.declare_dram_parameter("crep", [P, HGROUP * D], f32,
                                      isOutput=False)
    wh = nc.declare_dram_parameter("wvec", [P, 1], f32, isOutput=False)
    bh = nc.declare_dram_parameter("bvec", [P, 1], f32, isOutput=False)
    outh = nc.declare_dram_parameter("out", [n], f32, isOutput=True)

    x_v = xh[:, :].rearrange("(t p) k -> p t k", p=P)  # [128, tiles, 128]

    with ExitStack() as ctx:
        tc = ctx.enter_context(tile.TileContext(nc))
        singles = ctx.enter_context(tc.tile_pool(name="singles", bufs=1))
        xin = ctx.enter_context(tc.tile_pool(name="xin", bufs=xin_bufs))
        yp = ctx.enter_context(tc.tile_pool(name="y", bufs=y_bufs))
        dfp = ctx.enter_context(tc.tile_pool(name="df", bufs=3))
        vsp = ctx.enter_context(tc.tile_pool(name="vs", bufs=2))
        resp = ctx.enter_context(tc.tile_pool(name="res", bufs=2))
        rtp = ctx.enter_context(tc.tile_pool(name="rt", bufs=3))
        trp = ctx.enter_context(tc.tile_pool(name="tr", bufs=tr_bufs,
                                             space="PSUM"))
        pop = ctx.enter_context(tc.tile_pool(name="po", bufs=2, space="PSUM"))
        ttp = ctx.enter_context(tc.tile_pool(name="tt", bufs=2, space="PSUM"))

        negc_s = singles.tile([P, 1], f32)
        nc.sync.dma_start(out=negc_s, in_=negch[:, :])
        ident_s = singles.tile([P, D], f32)
        nc.sync.dma_start(out=ident_s, in_=identh[:, :])
        ones_s = singles.tile([P, 2], f32)
        nc.sync.dma_start(out=ones_s, in_=onesh[:, :])
        ones_r = singles.tile([P, 2], f32r)
        nc.vector.tensor_copy(out=ones_r, in_=ones_s)
        crep_s = singles.tile([P, HGROUP * D], f32)
        nc.sync.dma_start(out=crep_s, in_=creph[:, :])
        crep3 = crep_s.rearrange("p (t k) -> p t k", k=D)
        wv_s = singles.tile([P, 1], f32)
        nc.sync.dma_start(out=wv_s, in_=wh[:, :])
        bv_s = singles.tile([P, 1], f32)
        nc.sync.dma_start(out=bv_s, in_=bh[:, :])

        pe_acc = {"buf": None, "tiles": []}
        v_acc = {"buf": None, "tiles": []}

        def finalize(acc, kind):
            buf, tlist = acc["buf"], acc["tiles"]
            T = len(tlist)
            stride = 2 if kind == "pe" else 1
            C = stride * T
            res = resp.tile([P, 2 * po_tiles], f32, name="res", tag="res")
            nc.scalar.activation(out=res[:, :C], in_=buf[:, :C],
                                 func=Act.Exp, scale=-GAMMA, bias=0.0)
            nc.vector.tensor_scalar(out=res[:, :C], in0=res[:, :C],
                                    scalar1=wv_s[:, :], scalar2=bv_s[:, :],
                                    op0=Alu.mult, op1=Alu.add)
            c0 = 0
            while c0 < C:
                ncol = min(P, C - c0)
                nt = ncol // stride
                t0 = c0 // stride
                tt = ttp.tile([P, D], f32, name="tt", tag="tt")
                nc.tensor.transpose(out=tt[:ncol, :],
                                    in_=res[:, c0:c0 + ncol],
                                    identity=ident_s[:, :])
                rt = rtp.tile([P, D], f32, name="rt", tag="rt")
                nc.vector.tensor_copy(out=rt[:ncol, :], in_=tt[:ncol, :])
                if kind == "pe":
                    rtv = rt.rearrange("(t two) f -> t two f", two=2)
                else:
                    rtv = None
                # one DMA per run of consecutive global tiles
                li = 0
                while li < nt:
                    lj = li + 1
                    while (lj < nt
                           and tlist[t0 + lj] == tlist[t0 + lj - 1] + 1):
                        lj += 1
                    L = lj - li
                    tg = tlist[t0 + li]
                    src = (rtv[li:lj, 0, :] if kind == "pe"
                           else rt[li:lj, :])
                    dest = outh[tg * P:(tg + L) * P].rearrange(
                        "(t p) -> t p", p=P)
                    nc.sync.dma_start(out=dest, in_=src)
                    li = lj
                c0 += ncol
            acc["buf"] = None
            acc["tiles"] = []

        def pe_half(xt, hg, ht, t_base):
            tr = trp.tile([P, HGROUP * D], f32, name="tr", tag="tr")
            for j in range(ht):
                nc.tensor.transpose(out=tr[:, j * D:(j + 1) * D],
                                    in_=xt[:, hg + j, :],
                                    identity=ident_s[:, :])
            y = yp.tile([P, HGROUP * D], f32r, name="y", tag="y")
            nc.scalar.activation(out=y[:, :ht * D], in_=tr[:, :ht * D],
                                 func=Act.Square, bias=negc_s[:, :],
                                 scale=1.0)
            if stage == "sq":
                return
            for j in range(ht):
                if pe_acc["buf"] is None:
                    pe_acc["buf"] = pop.tile([P, 2 * po_tiles], f32,
                                             name="po", tag="po")
                    pe_acc["tiles"] = []
                col = 2 * len(pe_acc["tiles"])
                nc.tensor.matmul(out=pe_acc["buf"][:, col:col + 2],
                                 lhsT=y[:, j * D:(j + 1) * D],
                                 rhs=ones_r[:, :], start=True, stop=True)
                pe_acc["tiles"].append(t_base + j)
                if len(pe_acc["tiles"]) == po_tiles:
                    finalize(pe_acc, "pe")

        def v_half(xt, hg, ht, t_base):
            df = dfp.tile([P, HGROUP, D], f32, name="df", tag="df")
            nc.vector.tensor_sub(out=df[:, :ht, :], in0=xt[:, hg:hg + ht, :],
                                 in1=crep3[:, :ht, :])
            nc.scalar.activation(out=df[:, :ht, :], in_=df[:, :ht, :],
                                 func=Act.Square, bias=0.0, scale=1.0)
            if stage == "sq":
                return
            if v_acc["buf"] is None:
                v_acc["buf"] = vsp.tile([P, po_tiles], f32, name="vs",
                                        tag="vs")
                v_acc["tiles"] = []
            c0 = len(v_acc["tiles"])
            nc.vector.tensor_reduce(out=v_acc["buf"][:, c0:c0 + ht],
                                    in_=df[:, :ht, :],
                                    axis=mybir.AxisListType.X, op=Alu.add)
            v_acc["tiles"].extend(t_base + j for j in range(ht))
            if len(v_acc["tiles"]) + HGROUP > po_tiles:
                finalize(v_acc, "v")

        for _rep in range(repeats):
            hg_idx = 0
            t_done = 0
            while t_done < tiles:
                gt = min(group, tiles - t_done)
                xt = xin.tile([P, group, D], f32, name="xt", tag="xt")
                nc.sync.dma_start(out=xt[:, :gt, :],
                                  in_=x_v[:, t_done:t_done + gt, :])
                hg = 0
                while hg < gt and stage != "dma":
                    ht = min(HGROUP, gt - hg)
                    is_pe = (((hg_idx + 1) * pe_num) // den
                             > (hg_idx * pe_num) // den)
                    if is_pe:
                        pe_half(xt, hg, ht, t_done + hg)
                    else:
                        v_half(xt, hg, ht, t_done + hg)
                    hg_idx += 1
                    hg += ht
                t_done += gt
            if pe_acc["buf"] is not None:
                finalize(pe_acc, "pe")
            if v_acc["buf"] is not None:
                finalize(v_acc, "v")
        if stage != "full":
            # keep the output tensor written so the NEFF has a producer
            dest = outh[0:P].rearrange("(t p) -> t p", p=P)
            nc.sync.dma_start(out=dest, in_=ident_s[0:1, :])

    nc.finalize()
    return nc


def _get_nc(tiles=TILES):
    if tiles not in _NC_CACHE:
        _NC_CACHE[tiles] = _build(tiles)
    return _NC_CACHE[tiles]


def _make_const_inputs(centroid, w, b):
    centroid = np.asarray(centroid, dtype=np.float32).reshape(D)
    w = np.asarray(w, dtype=np.float32).reshape(-1)[0]
    b = np.asarray(b, dtype=np.float32).reshape(-1)[0]
    return {
        "negc": (-centroid).reshape(P, 1).copy(),
        "ident": np.eye(P, dtype=np.float32),
        "ones": np.tile(np.array([1.0, 0.0], dtype=np.float32), (P, 1)),
        "crep": np.tile(np.tile(centroid, HGROUP), (P, 1)),
        "wvec": np.full((P, 1), w, dtype=np.float32),
        "bvec": np.full((P, 1), b, dtype=np.float32),
    }


def kernel(X, centroid, w, b, _trace=False, _trace_kwargs=None):
    from concourse.bass_utils import run_bass_kernel_spmd

    X = np.asarray(X)
    assert X.shape == (N_TOTAL, D), X.shape
    if X.dtype != np.float32:
        X = X.astype(np.float32)

    consts = _make_const_inputs(centroid, w, b)
    starts = [i * SHARD for i in range(N_CORES - 1)] + [N_TOTAL - SHARD]
    in_maps = [dict(consts, x=X[s:s + SHARD]) for s in starts]

    nc = _get_nc()
    kw = {}
    if _trace:
        kw = dict(trace=True, **(_trace_kwargs or {}))
    res = run_bass_kernel_spmd(nc, in_maps, list(range(N_CORES)), **kw)

    out = np.empty(N_TOTAL, dtype=np.float32)
    for i, s in enumerate(starts):
        out[s:s + SHARD] = res.results[i]["out"]
    if _trace:
        return out, res
    return out


# revision 27
# speedup vs baseline: 1.0570x; 1.0570x over previous
"""Trainium2 Bass kernel for CustomRBF forward:

    out[i] = w * exp(-gamma * ||X[i] - centroid||^2) + b

Hybrid dual-path design (per core, data-parallel over 8 cores):
  - DMA X in natural layout [128 samples (partitions), 128 feats (free)],
    16 sample-tiles (1 MB) per dma_start.
  - Half-groups of 8 tiles alternate between two compute paths so TensorE
    and VectorE each carry ~half the per-sample reduction load in parallel:
    * PE path: TensorE transpose each tile to PSUM [feat, sample]; ScalarE
      fused subtract+square (activation Square, bias=-c, per-partition =
      per-feature); TensorE fp32r matmul (squared tile stationary, [1,0]
      moving) reduces over partitions -> 2 PSUM columns per tile ([sum, 0])
      in a [128, 512] accumulator.
    * DVE path (natural layout): VectorE tensor_sub against a replicated
      centroid row, ScalarE Square, VectorE segmented tensor_reduce over
      the feature axis -> [128, 8] columns in an SBUF [128, 256] accumulator.
  - Finalize per accumulator: ScalarE Exp (scale=-gamma), VectorE
    tensor_scalar (*w + b), TensorE transpose-back in 128-col chunks,
    VectorE PSUM->SBUF copy, then one output DMA per run of consecutive
    tiles (each path owns interleaved 8-tile blocks).

Sharding: cores 0-6 take contiguous 125056-sample slices; core 7 takes the
last 125056 samples (overlapping core 6 by 448 samples so every core gets
exactly 977 full 128-sample tiles). The overlap is recomputed identically
and overwritten at gather time.

`repeats` re-emits the whole pipeline R times in one NEFF (same data, same
output) — used only for differential wall-clock timing of the steady state.
"""

import sys

sys.path.insert(0, "/opt/trn_rl_repo")

import numpy as np

D = 128          # feature dim
P = 128          # SBUF partitions
GAMMA = 1.0 / D
N_CORES = 8
TILES = 977      # 128-sample tiles per core
SHARD = TILES * P           # 125056
N_TOTAL = 1000000
GROUP = 16       # tiles per DMA
HGROUP = 8       # tiles per half-group
PO_TILES = 256   # tiles per accumulator

_NC_CACHE = {}


def _build(tiles=TILES, po_tiles=PO_TILES, repeats=1, group=GROUP,
           xin_bufs=4, y_bufs=3, tr_bufs=2, stage="full", pe_num=1, den=2):
    from contextlib import ExitStack

    import concourse.tile as tile
    from concourse import bacc, mybir

    f32 = mybir.dt.float32
    f32r = mybir.dt.float32r
    Act = mybir.ActivationFunctionType
    Alu = mybir.AluOpType

    n = tiles * P
    nc = bacc.Bacc("TRN2", target_bir_lowering=False, debug=False,
                   num_devices=N_CORES)
    xh = nc.declare_dram_parameter("x", [n, D], f32, isOutput=False)
    negch = nc.declare_dram_parameter("negc", [P, 1], f32, isOutput=False)
    identh = nc.declare_dram_parameter("ident", [P, D], f32, isOutput=False)
    onesh = nc.declare_dram_parameter("ones", [P, 2], f32, isOutput=False)
    creph = nc.declare_dram_parameter("crep", [P, HGROUP * D], f32,
                                      isOutput=False)
    wh = nc.declare_dram_parameter("wvec", [P, 1], f32, isOutput=False)
    bh = nc.declare_dram_parameter("bvec", [P, 1], f32, isOutput=False)
    outh = nc.declare_dram_parameter("out", [n], f32, isOutput=True)

    x_v = xh[:, :].rearrange("(t p) k -> p t k", p=P)  # [128, tiles, 128]

    with ExitStack() as ctx:
        tc = ctx.enter_context(tile.TileContext(nc))
        singles = ctx.enter_context(tc.tile_pool(name="singles", bufs=1))
        xin = ctx.enter_context(tc.tile_pool(name="xin", bufs=xin_bufs))
        yp = ctx.enter_context(tc.tile_pool(name="y", bufs=y_bufs))
        dfp = ctx.enter_context(tc.tile_pool(name="df", bufs=3))
        vsp = ctx.enter_context(tc.tile_pool(name="vs", bufs=2))
        resp = ctx.enter_context(tc.tile_pool(name="res", bufs=2))
        rtp = ctx.enter_context(tc.tile_pool(name="rt", bufs=3))
        trp = ctx.enter_context(tc.tile_pool(name="tr", bufs=tr_bufs,
                                             space="PSUM"))
        pop = ctx.enter_context(tc.tile_pool(name="po", bufs=2, space="PSUM"))
        ttp = ctx.enter_context(tc.tile_pool(name="tt", bufs=2, space="PSUM"))

        negc_s = singles.tile([P, 1], f32)
        nc.sync.dma_start(out=negc_s, in_=negch[:, :])
        ident_s = singles.tile([P, D], f32)
        nc.sync.dma_start(out=ident_s, in_=identh[:, :])
        ones_s = singles.tile([P, 2], f32)
        nc.sync.dma_start(out=ones_s, in_=onesh[:, :])
        ones_r = singles.tile([P, 2], f32r)
        nc.vector.tensor_copy(out=ones_r, in_=ones_s)
        crep_s = singles.tile([P, HGROUP * D], f32)
        nc.sync.dma_start(out=crep_s, in_=creph[:, :])
        crep3 = crep_s.rearrange("p (t k) -> p t k", k=D)
        wv_s = singles.tile([P, 1], f32)
        nc.sync.dma_start(out=wv_s, in_=wh[:, :])
        bv_s = singles.tile([P, 1], f32)
        nc.sync.dma_start(out=bv_s, in_=bh[:, :])

        pe_acc = {"buf": None, "tiles": []}
        v_acc = {"buf": None, "tiles": []}

        def finalize(acc, kind):
            buf, tlist = acc["buf"], acc["tiles"]
            T = len(tlist)
            stride = 2 if kind == "pe" else 1
            C = stride * T
            res = resp.tile([P, 2 * po_tiles], f32, name="res", tag="res")
            nc.scalar.activation(out=res[:, :C], in_=buf[:, :C],
                                 func=Act.Exp, scale=-GAMMA, bias=0.0)
            nc.vector.tensor_scalar(out=res[:, :C], in0=res[:, :C],
                                    scalar1=wv_s[:, :], scalar2=bv_s[:, :],
                                    op0=Alu.mult, op1=Alu.add)
            c0 = 0
            while c0 < C:
                ncol = min(P, C - c0)
                nt = ncol // stride
                t0 = c0 // stride
                tt = ttp.tile([P, D], f32, name="tt", tag="tt")
                nc.tensor.transpose(out=tt[:ncol, :],
                                    in_=res[:, c0:c0 + ncol],
                                    identity=ident_s[:, :])
                rt = rtp.tile([P, D], f32, name="rt", tag="rt")
                nc.vector.tensor_copy(out=rt[:ncol, :], in_=tt[:ncol, :])
                if kind == "pe":
                    rtv = rt.rearrange("(t two) f -> t two f", two=2)
                else:
                    rtv = None
                # one DMA per run of consecutive global tiles
                li = 0
                while li < nt:
                    lj = li + 1
                    while (lj < nt
                           and tlist[t0 + lj] == tlist[t0 + lj - 1] + 1):
                        lj += 1
                    L = lj - li
                    tg = tlist[t0 + li]
                    src = (rtv[li:lj, 0, :] if kind == "pe"
                           else rt[li:lj, :])
                    dest = outh[tg * P:(tg + L) * P].rearrange(
                        "(t p) -> t p", p=P)
                    nc.sync.dma_start(out=dest, in_=src)
                    li = lj
                c0 += ncol
            acc["buf"] = None
            acc["tiles"] = []

        # one-deep deferral per path: each halfgroup's reduce stage is
        # emitted when the NEXT halfgroup of that path arrives, so neither
        # engine stream stalls on the cross-engine square in between.
        pending = {"pe": None, "v": None}

        def flush(path):
            fn = pending[path]
            if fn is not None:
                pending[path] = None
                fn()

        def pe_half(xt, hg, ht, t_base):
            tr = trp.tile([P, HGROUP * D], f32, name="tr", tag="tr")
            for j in range(ht):
                nc.tensor.transpose(out=tr[:, j * D:(j + 1) * D],
                                    in_=xt[:, hg + j, :],
                                    identity=ident_s[:, :])
            y = yp.tile([P, HGROUP * D], f32r, name="y", tag="y")
            nc.scalar.activation(out=y[:, :ht * D], in_=tr[:, :ht * D],
                                 func=Act.Square, bias=negc_s[:, :],
                                 scale=1.0)
            if stage == "sq":
                return

            def back():
                for j in range(ht):
                    if pe_acc["buf"] is None:
                        pe_acc["buf"] = pop.tile([P, 2 * po_tiles], f32,
                                                 name="po", tag="po")
                        pe_acc["tiles"] = []
                    col = 2 * len(pe_acc["tiles"])
                    nc.tensor.matmul(out=pe_acc["buf"][:, col:col + 2],
                                     lhsT=y[:, j * D:(j + 1) * D],
                                     rhs=ones_r[:, :], start=True, stop=True)
                    pe_acc["tiles"].append(t_base + j)
                    if len(pe_acc["tiles"]) == po_tiles:
                        finalize(pe_acc, "pe")

            pending["pe"] = back

        def v_half(xt, hg, ht, t_base):
            df = dfp.tile([P, HGROUP, D], f32, name="df", tag="df")
            nc.vector.tensor_sub(out=df[:, :ht, :], in0=xt[:, hg:hg + ht, :],
                                 in1=crep3[:, :ht, :])
            nc.scalar.activation(out=df[:, :ht, :], in_=df[:, :ht, :],
                                 func=Act.Square, bias=0.0, scale=1.0)
            if stage == "sq":
                return

            def back():
                if v_acc["buf"] is None:
                    v_acc["buf"] = vsp.tile([P, po_tiles], f32, name="vs",
                                            tag="vs")
                    v_acc["tiles"] = []
                c0 = len(v_acc["tiles"])
                nc.vector.tensor_reduce(out=v_acc["buf"][:, c0:c0 + ht],
                                        in_=df[:, :ht, :],
                                        axis=mybir.AxisListType.X,
                                        op=Alu.add)
                v_acc["tiles"].extend(t_base + j for j in range(ht))
                if len(v_acc["tiles"]) + HGROUP > po_tiles:
                    finalize(v_acc, "v")

            pending["v"] = back

        for _rep in range(repeats):
            hg_idx = 0
            t_done = 0
            while t_done < tiles:
                gt = min(group, tiles - t_done)
                xt = xin.tile([P, group, D], f32, name="xt", tag="xt")
                nc.sync.dma_start(out=xt[:, :gt, :],
                                  in_=x_v[:, t_done:t_done + gt, :])
                hg = 0
                while hg < gt and stage != "dma":
                    ht = min(HGROUP, gt - hg)
                    is_pe = (((hg_idx + 1) * pe_num) // den
                             > (hg_idx * pe_num) // den)
                    if is_pe:
                        flush("pe")
                        pe_half(xt, hg, ht, t_done + hg)
                    else:
                        flush("v")
                        v_half(xt, hg, ht, t_done + hg)
                    hg_idx += 1
                    hg += ht
                t_done += gt
            flush("pe")
            flush("v")
            if pe_acc["buf"] is not None:
                finalize(pe_acc, "pe")
            if v_acc["buf"] is not None:
                finalize(v_acc, "v")
        if stage != "full":
            # keep the output tensor written so the NEFF has a producer
            dest = outh[0:P].rearrange("(t p) -> t p", p=P)
            nc.sync.dma_start(out=dest, in_=ident_s[0:1, :])

    nc.finalize()
    return nc


def _get_nc(tiles=TILES):
    if tiles not in _NC_CACHE:
        _NC_CACHE[tiles] = _build(tiles)
    return _NC_CACHE[tiles]


def _make_const_inputs(centroid, w, b):
    centroid = np.asarray(centroid, dtype=np.float32).reshape(D)
    w = np.asarray(w, dtype=np.float32).reshape(-1)[0]
    b = np.asarray(b, dtype=np.float32).reshape(-1)[0]
    return {
        "negc": (-centroid).reshape(P, 1).copy(),
        "ident": np.eye(P, dtype=np.float32),
        "ones": np.tile(np.array([1.0, 0.0], dtype=np.float32), (P, 1)),
        "crep": np.tile(np.tile(centroid, HGROUP), (P, 1)),
        "wvec": np.full((P, 1), w, dtype=np.float32),
        "bvec": np.full((P, 1), b, dtype=np.float32),
    }


def kernel(X, centroid, w, b, _trace=False, _trace_kwargs=None):
    from concourse.bass_utils import run_bass_kernel_spmd

    X = np.asarray(X)
    assert X.shape == (N_TOTAL, D), X.shape
    if X.dtype != np.float32:
        X = X.astype(np.float32)

    consts = _make_const_inputs(centroid, w, b)
    starts = [i * SHARD for i in range(N_CORES - 1)] + [N_TOTAL - SHARD]
    in_maps = [dict(consts, x=X[s:s + SHARD]) for s in starts]

    nc = _get_nc()
    kw = {}
    if _trace:
        kw = dict(trace=True, **(_trace_kwargs or {}))
    res = run_bass_kernel_spmd(nc, in_maps, list(range(N_CORES)), **kw)

    out = np.empty(N_TOTAL, dtype=np.float32)
    for i, s in enumerate(starts):
        out[s:s + SHARD] = res.results[i]["out"]
    if _trace:
        return out, res
    return out
